# revision 23
# baseline (speedup 1.0000x reference)
"""GRACE contrastive loss kernel for Trainium2 (8 NeuronCores, SPMD).

Strategy (symmetric row-block data parallel, fp8 everywhere):
  - Shard the N=8192 nodes across 8 cores (1024 rows each).  Projection
    MLP runs in fp8 DoubleRow (weights + activations quantized, ELU's
    "-1" folded into b2 host-side), fp32 accumulation; per-node 1/norms
    via exp(-0.5*ln(sum h^2)); normalized embeddings quantized to fp8
    and AllGather'd per view (a tiny dummy AllGather issued at t=0
    pulls the collective entry barrier off the critical path).
  - S11/S22 are symmetric: each core computes only shifts d=0..4 of its
    block row (d = (col_block - core) mod 8); row sums of the computed
    exp-blocks cover d=0..4, the d=5..7 contributions arrive as column
    sums computed by neighbor cores (shift 8-d in {1,2,3}), routed via
    one fp32 ReduceScatter.  The d=4 block is computed redundantly by
    both cores of its pair (no colsum exchange) to keep the SPMD
    program uniform.  S12 is computed in full per row; its column sums
    (= S21 row sums) ride the same ReduceScatter.
  - The gathered embeddings are loaded into SBUF in *rotated* order
    (slot d holds global block (core+d) mod 8) using dynamic-offset
    DMAs driven by the partition id, so all matmul addressing is
    uniform across cores.  Shift-0 (diagonal) blocks use the local
    embeddings directly and run while the AllGathers are in flight.
  - Sim groups are fp8 DoubleRow matmuls (N=1024 moving operand) into
    [128 x 2048] (or 1024) PSUM groups with fused exp(2s) + row-sum
    (accum_out) on the scalar engine; exp tiles for colsum-contributing
    blocks accumulate into fp16 buffers (DVE), partition-reduced by
    ones-matmuls and scattered into the ReduceScatter input at
    dynamically-computed (core-relative) offsets.  All Ln's run at the
    very end so the ACT exp table is loaded only once per set.
  - Per-core scalar partial out; host sums partials / N.
"""

import math
import sys

import numpy as np

sys.path.insert(0, "/opt/trn_rl_repo")

import concourse.bass as bass  # noqa: E402
import concourse.mybir as mybir  # noqa: E402
import concourse.tile as tile  # noqa: E402
from concourse import bacc  # noqa: E402
from concourse.bass_utils import run_bass_kernel_spmd  # noqa: E402

F32 = mybir.dt.float32
F32R = mybir.dt.float32r
F16 = mybir.dt.float16
F8 = mybir.dt.float8e4
AF = mybir.ActivationFunctionType
ALU = mybir.AluOpType
DR = mybir.MatmulPerfMode.DoubleRow

N_CORES = 8
N = 8192
D = 512            # feature dim (= H = P in the reference MLP)
NB = N // N_CORES  # 1024 rows per core
KT = D // 128      # 4 k-subtiles
MT = NB // 128     # 8 row tiles per core
NCHUNK = 512       # projection matmul moving width
SIMW = 1024        # sim matmul moving width (max for fp8)
BLK = D * NB       # elements in one gathered fp8 block
TAU_INV = 2.0      # 1 / tau
E2 = float(np.exp(2.0, dtype=np.float64))

TRACE = False
LAST_EXEC_NS = None
_CACHE = {}


def _build_program(sim_mode=False):
    nc = bacc.Bacc("TRN2", target_bir_lowering=False, debug=False,
                   num_devices=N_CORES)

    # ---- I/O ----
    zt1 = nc.dram_tensor("zt1", [128, KT, NB], F8, kind="ExternalInput").ap()
    zt2 = nc.dram_tensor("zt2", [128, KT, NB], F8, kind="ExternalInput").ap()
    w1t = nc.dram_tensor("w1t", [128, KT, D], F8, kind="ExternalInput").ap()
    w2t = nc.dram_tensor("w2t", [128, KT, D], F8, kind="ExternalInput").ap()
    b1 = nc.dram_tensor("b1", [128, KT], F32, kind="ExternalInput").ap()
    b2p = nc.dram_tensor("b2p", [128, KT], F32, kind="ExternalInput").ap()
    out = nc.dram_tensor("out", [1, 1], F32, kind="ExternalOutput").ap()

    rg = [list(range(N_CORES))]

    with tile.TileContext(nc) as tc:
        with tc.tile_pool(name="persist", bufs=1) as persist, \
             tc.tile_pool(name="dram", bufs=1, space="DRAM") as dram, \
             tc.tile_pool(name="stats", bufs=1) as stats:

            ones_cs = persist.tile([128, 1], F32)
            nc.vector.memset(ones_cs[:], 1.0)
            ones_col = persist.tile([128, 1], F32R)
            nc.vector.tensor_copy(ones_col[:], ones_cs[:])
            ones_sc = persist.tile([1, 128], F32)
            nc.vector.memset(ones_sc[:], 1.0)
            ones_row = persist.tile([1, 128], F32R)
            nc.vector.tensor_copy(ones_row[:], ones_sc[:])
            ones_16 = persist.tile([128, 1], F16)
            nc.vector.memset(ones_16[:], 1.0)

            # normalized fp8 local blocks [feature, node] (sims lhsT + d0 rhs)
            n8 = [persist.tile([128, KT, NB], F8, name=f"n8_{v}")
                  for v in range(2)]
            rn_vec = [persist.tile([1, NB], F32R, name=f"rn{v}") for v in range(2)]
            # rotated gathered embeddings: slot di holds global block
            # (core + di + 1) mod 8; [128, slot, KT, NB] keeps every
            # DMA line contiguous (4KB per partition per slot)
            g1 = persist.tile([128, 4, KT, NB], F8, name="g1")
            g2 = persist.tile([128, 7, KT, NB], F8, name="g2")
            # colsum accumulators (rotated slot order)
            acc11 = persist.tile([128, 3 * NB], F16, name="acc11")
            acc22 = persist.tile([128, 3 * NB], F16, name="acc22")
            acc12 = persist.tile([128, 8 * NB], F16, name="acc12")

            # DRAM buffers
            shr = {} if sim_mode else {"addr_space": "Shared"}
            db_in = dram.tile([1, 8], F32, name="db_in")
            db_out = dram.tile([1, 64], F32, name="db_out",
                               tag="dbbuf", **shr)
            cc_in = dram.tile([2 * 128, KT * NB], F8, name="cc_in")
            ccf = dram.tile([1, N_CORES * 2 * BLK], F8, name="cc_out",
                            tag="agbuf0", **shr)
            cs_in = dram.tile([1, N_CORES * 3 * NB], F32, name="cs_in")
            cs_out = dram.tile([3 * NB], F32, name="cs_out")

            pos_part = stats.tile([1, NB], F32, name="pos_part")
            # row-sum partials: [128, MT, slots]
            parts11 = stats.tile([128, MT, 4], F32, name="parts11")
            parts12 = stats.tile([128, MT, 4], F32, name="parts12")
            parts22 = stats.tile([128, MT, 4], F32, name="parts22")
            pos_sum = stats.tile([1, 1], F32)
            lnwarm = stats.tile([1, 1], F32, name="lnwarm")

            # dummy collective: absorbs the entry barrier + first-collective
            # stream costs so AG1 runs fast (measured 24us vs 33us without)
            zcs = persist.tile([1, 3 * NB], F32, name="zcs")
            nc.vector.memset(zcs[:], 0.0)
            nc.sync.dma_start(db_in[:], zcs[:, 0:8])
            if not sim_mode:
                nc.gpsimd.collective_compute(
                    "AllGather", ALU.bypass, replica_groups=rg,
                    ins=[db_in.opt()], outs=[db_out.opt()])
            # zero-init the ReduceScatter input (S11/S22 sections of
            # slots this core does not write must contribute zero)
            for j in range(N_CORES):
                nc.sync.dma_start(cs_in[:, j * 3 * NB:(j + 1) * 3 * NB], zcs[:])

            # ---------------- projection phase ----------------
            with tc.tile_pool(name="proj", bufs=1) as proj, \
                 tc.tile_pool(name="ptmp", bufs=2) as ptmp, \
                 tc.tile_pool(name="ppsum", bufs=4, space="PSUM") as ppsum, \
                 tc.tile_pool(name="spsum", bufs=2, space="PSUM") as spsum:

                zt_sb = [proj.tile([128, KT, NB], F8, name=f"zt_sb{v}")
                         for v in range(2)]
                w1_sb = proj.tile([128, KT, D], F8)
                w2_sb = proj.tile([128, KT, D], F8)
                b1_sb = proj.tile([128, KT], F32)
                b2_sb = proj.tile([128, KT], F32)
                u_sb = proj.tile([128, KT, NB], F8)   # ELU out + 1
                h_sb = [proj.tile([128, KT, NB], F32, name=f"h{v}")
                        for v in range(2)]
                hsq = proj.tile([128, KT, NB], F32R)

                nc.sync.dma_start(w1_sb[:], w1t)
                nc.sync.dma_start(b1_sb[:], b1)
                nc.sync.dma_start(zt_sb[0][:], zt1)
                nc.sync.dma_start(w2_sb[:], w2t)
                nc.sync.dma_start(b2_sb[:], b2p)
                nc.sync.dma_start(zt_sb[1][:], zt2)

                for v in range(2):
                    # ---- layer 1 + ELU (u = elu(y) + 1 >= 0) ----
                    for pt in range(KT):
                        for ch in range(NB // NCHUNK):
                            csl = slice(ch * NCHUNK, (ch + 1) * NCHUNK)
                            ps = ppsum.tile([128, NCHUNK], F32, tag="ps_proj")
                            for k2 in range(KT // 2):
                                nc.tensor.matmul(
                                    ps[:],
                                    lhsT=w1_sb[:, 2 * k2:2 * k2 + 2,
                                               pt * 128:(pt + 1) * 128],
                                    rhs=zt_sb[v][:, 2 * k2:2 * k2 + 2, csl],
                                    start=(k2 == 0), stop=(k2 == KT // 2 - 1),
                                    perf_mode=DR)
                            texp = ptmp.tile([128, NCHUNK], F16, tag="texp")
                            nc.scalar.activation(texp[:], ps[:], AF.Exp,
                                                 bias=b1_sb[:, pt:pt + 1],
                                                 scale=1.0)
                            tmax = ptmp.tile([128, NCHUNK], F16, tag="tmax")
                            nc.scalar.activation(tmax[:], ps[:], AF.Relu,
                                                 bias=b1_sb[:, pt:pt + 1],
                                                 scale=1.0)
                            # u = min(exp(y),1) + relu(y)
                            nc.vector.scalar_tensor_tensor(
                                u_sb[:, pt, csl], texp[:], 1.0, tmax[:],
                                ALU.min, ALU.add)
                    # ---- layer 2 (+ folded b2) + squares ----
                    for jt in range(KT):
                        for ch in range(NB // NCHUNK):
                            csl = slice(ch * NCHUNK, (ch + 1) * NCHUNK)
                            ps = ppsum.tile([128, NCHUNK], F32, tag="ps_proj")
                            for k2 in range(KT // 2):
                                nc.tensor.matmul(
                                    ps[:],
                                    lhsT=w2_sb[:, 2 * k2:2 * k2 + 2,
                                               jt * 128:(jt + 1) * 128],
                                    rhs=u_sb[:, 2 * k2:2 * k2 + 2, csl],
                                    start=(k2 == 0), stop=(k2 == KT // 2 - 1),
                                    perf_mode=DR)
                            sl = (slice(None), jt, csl)
                            nc.vector.tensor_scalar(h_sb[v][sl], ps[:],
                                                    b2_sb[:, jt:jt + 1], None,
                                                    ALU.add)
                            nc.scalar.activation(hsq[sl], h_sb[v][sl], AF.Square)
                    # ---- 1/norm: rn = exp(-0.5*ln(ss)); Ln's batched ----
                    tlns = []
                    for ch in range(NB // NCHUNK):
                        csl = slice(ch * NCHUNK, (ch + 1) * NCHUNK)
                        pss = spsum.tile([1, NCHUNK], F32, tag="ps_small")
                        for jt in range(KT):
                            nc.tensor.matmul(
                                pss[:],
                                lhsT=ones_col[:],
                                rhs=hsq[:, jt, csl],
                                start=(jt == 0), stop=(jt == KT - 1))
                        tln = ptmp.tile([1, NCHUNK], F32, tag="tln")
                        nc.scalar.activation(tln[:], pss[:], AF.Ln)
                        tlns.append(tln)
                    for ch in range(NB // NCHUNK):
                        csl = slice(ch * NCHUNK, (ch + 1) * NCHUNK)
                        nc.scalar.activation(rn_vec[v][:, csl], tlns[ch][:],
                                             AF.Exp, scale=-0.5)
                    # ---- normalize + quantize to fp8 ----
                    for ch in range(NB // NCHUNK):
                        csl = slice(ch * NCHUNK, (ch + 1) * NCHUNK)
                        pbc = spsum.tile([128, NCHUNK], F32, tag="ps_bc")
                        nc.tensor.matmul(
                            pbc[:], lhsT=ones_row[:],
                            rhs=rn_vec[v][:, csl],
                            start=True, stop=True)
                        for jt in range(KT):
                            nc.vector.tensor_tensor(
                                n8[v][:, jt, csl], h_sb[v][:, jt, csl],
                                pbc[:], ALU.mult)
                    # ---- ship; one merged AllGather after both views ----
                    for hf in range(2):
                        nc.sync.dma_start(
                            cc_in[v * 128:(v + 1) * 128,
                                  hf * 2 * NB:(hf + 1) * 2 * NB],
                            n8[v][:, 2 * hf:2 * hf + 2, :])
                    if v == 1:
                        if sim_mode:
                            for r in range(N_CORES):
                                nc.sync.dma_start(
                                    ccf[:, r * 2 * BLK:(r + 1) * 2 * BLK],
                                    cc_in.rearrange("p m -> (p m)"))
                        else:
                            nc.gpsimd.collective_compute(
                                "AllGather", ALU.bypass, replica_groups=rg,
                                ins=[cc_in.opt()], outs=[ccf.opt()])

                # ---- pos diagonal: s12_ii = rn1_i*rn2_i*sum_f h1[f,i]h2[f,i]
                hh = hsq  # reuse
                for jt in range(KT):
                    nc.vector.tensor_tensor(hh[:, jt, :], h_sb[0][:, jt, :],
                                            h_sb[1][:, jt, :], ALU.mult)
                for ch in range(NB // NCHUNK):
                    csl = slice(ch * NCHUNK, (ch + 1) * NCHUNK)
                    psp = spsum.tile([1, NCHUNK], F32, tag="ps_small")
                    for jt in range(KT):
                        nc.tensor.matmul(psp[:],
                                         lhsT=ones_col[:],
                                         rhs=hh[:, jt, csl],
                                         start=(jt == 0), stop=(jt == KT - 1))
                    nc.vector.tensor_tensor(pos_part[:, csl], psp[:],
                                            rn_vec[0][:, csl], ALU.mult)
                    nc.vector.tensor_tensor(pos_part[:, csl], pos_part[:, csl],
                                            rn_vec[1][:, csl], ALU.mult)
                nc.vector.tensor_reduce(pos_sum[:], pos_part[:],
                                        mybir.AxisListType.X, ALU.add)

            # ---------------- rotated gathered loads ----------------
            pid = nc.sync.partition_id()
            for d in range(1, 5):
                off = ((pid + d) % N_CORES) * (2 * BLK)
                nc.sync.dma_start(
                    g1[:, d - 1, :, :],
                    ccf[:, bass.ds(off, BLK)]
                    .rearrange("o (p m) -> (o p) m", p=128))
            for d in range(1, 8):
                off = ((pid + d) % N_CORES) * (2 * BLK) + BLK
                nc.sync.dma_start(
                    g2[:, d - 1, :, :],
                    ccf[:, bass.ds(off, BLK)]
                    .rearrange("o (p m) -> (o p) m", p=128))

            # ---------------- similarity passes ----------------
            # chunks: list of (lhs_view, rhs3d) where rhs3d is [128, KT, NB]
            # flush: (acc, aoff, sec, d) — partition-reduce one NB-wide acc
            # column into the unused upper half of a 1024-wide group's psum
            # slot, then scatter it into cs_in at the core-relative slot.
            def sim_group(sp, scr, cs_sbp, mt, width, chunks, parts_ap,
                          acc_list, first_touch, dve_rowsum=False, flush=None):
                pss = sp.tile([128, 2048], F32, tag="ps_sim")
                for ci, (lv, r3) in enumerate(chunks):
                    for k2 in range(KT // 2):
                        for sc in range(SIMW // NCHUNK):
                            c0 = sc * NCHUNK
                            o0 = ci * SIMW + sc * NCHUNK
                            nc.tensor.matmul(
                                pss[:, o0:o0 + NCHUNK],
                                lhsT=n8[lv][:, 2 * k2:2 * k2 + 2,
                                            mt * 128:(mt + 1) * 128],
                                rhs=r3[:, 2 * k2:2 * k2 + 2, c0:c0 + NCHUNK],
                                start=(k2 == 0), stop=(k2 == KT // 2 - 1),
                                perf_mode=DR)
                if flush is not None:
                    facc, aoff, sec, dd = flush
                    for sc in range(NB // NCHUNK):
                        nc.tensor.matmul(
                            pss[0:1, 1024 + sc * NCHUNK:1024 + (sc + 1) * NCHUNK],
                            lhsT=ones_16[:],
                            rhs=facc[:, aoff + sc * NCHUNK:aoff + (sc + 1) * NCHUNK],
                            start=True, stop=True)
                    cst = cs_sbp.tile([1, NB], F32, tag="cs")
                    nc.vector.tensor_copy(cst[:], pss[0:1, 1024:2048])
                    woff = ((pid + dd) % N_CORES) * (3 * NB) + sec * NB
                    nc.sync.dma_start(cs_in[:, bass.ds(woff, NB)], cst[:])
                es = scr.tile([128, width], F16,
                              tag=("es2" if width == 2048 else "es1"))
                if dve_rowsum:
                    nc.scalar.activation(es[:], pss[:, 0:width], AF.Exp,
                                         scale=TAU_INV)
                    nc.vector.tensor_reduce(parts_ap, es[:],
                                            mybir.AxisListType.X, ALU.add)
                else:
                    nc.scalar.activation(es[:], pss[:, 0:width], AF.Exp,
                                         scale=TAU_INV, accum_out=parts_ap)
                for (acc, aoff, eoff, w2) in acc_list:
                    asl = acc[:, aoff:aoff + w2]
                    esl = es[:, eoff:eoff + w2]
                    if first_touch:
                        nc.vector.tensor_copy(asl, esl)
                    else:
                        nc.vector.tensor_tensor(asl, asl, esl, ALU.add)

            with tc.tile_pool(name="sim_psum", bufs=2, space="PSUM") as sp, \
                 tc.tile_pool(name="scr", bufs=4) as scr, \
                 tc.tile_pool(name="cs_sbp", bufs=4) as cs_sbp:

                # P1: shift-0 diagonal blocks (local, overlap AllGathers).
                # [S11d0|S12d0] share one group (both feed denom1 rows);
                # row sums on DVE (idle during the AllGather window).
                for mt in range(MT):
                    ft = (mt == 0)
                    sim_group(sp, scr, cs_sbp, mt, 2048,
                              [(0, n8[0]), (0, n8[1])],
                              parts11[:, mt, 0:1], [(acc12, 0, NB, NB)], ft,
                              dve_rowsum=True)
                    sim_group(sp, scr, cs_sbp, mt, 1024, [(1, n8[1])],
                              parts22[:, mt, 0:1], [], ft, dve_rowsum=True)

                # P2: S11 shifts 1..3 (needs g1)
                for mt in range(MT):
                    ft = (mt == 0)
                    sim_group(sp, scr, cs_sbp, mt, 2048,
                              [(0, g1[:, 0]), (0, g1[:, 1])],
                              parts11[:, mt, 1:2], [(acc11, 0, 0, 2 * NB)], ft)
                    sim_group(sp, scr, cs_sbp, mt, 1024, [(0, g1[:, 2])],
                              parts11[:, mt, 2:3], [(acc11, 2 * NB, 0, NB)],
                              ft, dve_rowsum=True)

                # P3: S12 shifts 1..7 (needs g2); acc11 colsums flush inside
                # the d7 groups' psum upper halves
                for mt in range(MT):
                    ft = (mt == 0)
                    sim_group(sp, scr, cs_sbp, mt, 2048,
                              [(0, g2[:, 0]), (0, g2[:, 1])],
                              parts12[:, mt, 0:1], [(acc12, NB, 0, 2 * NB)], ft)
                    sim_group(sp, scr, cs_sbp, mt, 2048,
                              [(0, g2[:, 2]), (0, g2[:, 3])],
                              parts12[:, mt, 1:2],
                              [(acc12, 3 * NB, 0, 2 * NB)], ft)
                    sim_group(sp, scr, cs_sbp, mt, 2048,
                              [(0, g2[:, 4]), (0, g2[:, 5])],
                              parts12[:, mt, 2:3],
                              [(acc12, 5 * NB, 0, 2 * NB)], ft)
                    fl = ((acc11, (mt - 1) * NB, 1, mt) if 1 <= mt <= 3
                          else None)
                    sim_group(sp, scr, cs_sbp, mt, 1024, [(0, g2[:, 6])],
                              parts12[:, mt, 3:4], [(acc12, 7 * NB, 0, NB)],
                              ft, dve_rowsum=True, flush=fl)

                # P4: S22 shifts 1..3; acc12 colsums flush inside d3 groups
                for mt in range(MT):
                    ft = (mt == 0)
                    sim_group(sp, scr, cs_sbp, mt, 2048,
                              [(1, g2[:, 0]), (1, g2[:, 1])],
                              parts22[:, mt, 1:2], [(acc22, 0, 0, 2 * NB)], ft)
                    sim_group(sp, scr, cs_sbp, mt, 1024, [(1, g2[:, 2])],
                              parts22[:, mt, 2:3], [(acc22, 2 * NB, 0, NB)],
                              ft, flush=(acc12, mt * NB, 0, mt))

                # P5: redundant shift-4 blocks (no colsums); acc22 flushes in
                # the first three groups, then the ReduceScatter fires and
                # overlaps the rest of the pass
                for mt in range(MT):
                    fl = (acc22, (mt) * NB, 2, mt + 1) if mt <= 2 else None
                    sim_group(sp, scr, cs_sbp, mt, 1024, [(0, g1[:, 3])],
                              parts11[:, mt, 3:4], [], False, dve_rowsum=True)
                    sim_group(sp, scr, cs_sbp, mt, 1024, [(1, g2[:, 3])],
                              parts22[:, mt, 3:4], [], False, flush=fl)
                    if mt == 3:
                        if sim_mode:
                            nc.sync.dma_start(cs_out[:],
                                              cs_in[:, 0:3 * NB]
                                              .rearrange("o m -> (o m)"))
                        else:
                            nc.gpsimd.collective_compute(
                                "ReduceScatter", ALU.add, replica_groups=rg,
                                ins=[cs_in.opt()], outs=[cs_out.opt()])
                # warm the Ln table set while the ReduceScatter drains, so
                # the final assembly pays no ACT_TABLE_LOAD.  The warm input
                # is parts22*0+1 so it depends on the last sim exp (not
                # hoisted) and ln(1)=0 folds harmlessly into pos_sum.
                nc.vector.tensor_scalar(lnwarm[:], parts22[0:1, 7, 3:4],
                                        0.0, 1.0, ALU.mult, ALU.add)
                nc.scalar.activation(lnwarm[:], lnwarm[:], AF.Ln)
                nc.vector.tensor_tensor(pos_sum[:], pos_sum[:], lnwarm[:],
                                        ALU.add)

            # ---------------- assemble the loss ----------------
            with tc.tile_pool(name="fin", bufs=1) as fsb, \
                 tc.tile_pool(name="fin_psum", bufs=1, space="PSUM") as fp:
                rs11 = fsb.tile([128, MT], F32)
                rs12 = fsb.tile([128, MT], F32)
                rs22 = fsb.tile([128, MT], F32)
                nc.vector.tensor_reduce(rs11[:], parts11[:],
                                        mybir.AxisListType.X, ALU.add)
                nc.vector.tensor_reduce(rs12[:], parts12[:],
                                        mybir.AxisListType.X, ALU.add)
                nc.vector.tensor_reduce(rs22[:], parts22[:],
                                        mybir.AxisListType.X, ALU.add)
                r21 = fsb.tile([128, MT], F32)
                r11 = fsb.tile([128, MT], F32)
                r22 = fsb.tile([128, MT], F32)
                nc.sync.dma_start(
                    r21[:], cs_out[0:NB].rearrange("(mt p) -> p mt", p=128))
                nc.sync.dma_start(
                    r11[:], cs_out[NB:2 * NB].rearrange("(mt p) -> p mt", p=128))
                nc.sync.dma_start(
                    r22[:], cs_out[2 * NB:3 * NB]
                    .rearrange("(mt p) -> p mt", p=128))

                d1 = fsb.tile([128, MT], F32)
                nc.vector.tensor_tensor(d1[:], rs11[:], r11[:], ALU.add)
                nc.vector.tensor_tensor(d1[:], d1[:], rs12[:], ALU.add)
                nc.vector.tensor_scalar_add(d1[:], d1[:], -E2)
                d2 = fsb.tile([128, MT], F32)
                nc.vector.tensor_tensor(d2[:], rs22[:], r22[:], ALU.add)
                nc.vector.tensor_tensor(d2[:], d2[:], r21[:], ALU.add)
                nc.vector.tensor_scalar_add(d2[:], d2[:], -E2)
                nc.scalar.activation(d1[:], d1[:], AF.Ln)
                nc.scalar.activation(d2[:], d2[:], AF.Ln)
                lsum = fsb.tile([128, MT], F32)
                nc.vector.tensor_tensor(lsum[:], d1[:], d2[:], ALU.add)
                lrow = fsb.tile([128, 1], F32)
                nc.vector.tensor_reduce(lrow[:], lsum[:],
                                        mybir.AxisListType.X, ALU.add)
                pfin = fp.tile([1, 1], F32)
                nc.tensor.matmul(pfin[:], lhsT=ones_cs[:], rhs=lrow[:],
                                 start=True, stop=True)
                fin = fsb.tile([1, 1], F32)
                nc.vector.tensor_scalar_mul(fin[:], pfin[:], 0.5)
                p2 = fsb.tile([1, 1], F32)
                nc.vector.tensor_scalar_mul(p2[:], pos_sum[:], 2.0)
                nc.vector.tensor_tensor(fin[:], fin[:], p2[:], ALU.subtract)
                nc.sync.dma_start(out, fin[:])

    nc.compile()
    return nc


def _to_fp8(x):
    import ml_dtypes
    return np.asarray(x, dtype=ml_dtypes.float8_e4m3fn)


def _prep_inputs(z1, z2, fc1_w, fc1_b, fc2_w, fc2_b):
    """Host-side shard + layout prep. Returns in_maps for the 8 cores."""
    w1t = np.ascontiguousarray(fc1_w.T).reshape(KT, 128, D).transpose(1, 0, 2)
    w1t = _to_fp8(np.ascontiguousarray(w1t))
    w2t = np.ascontiguousarray(fc2_w.T).reshape(KT, 128, D).transpose(1, 0, 2)
    w2t = _to_fp8(np.ascontiguousarray(w2t))
    b1 = np.ascontiguousarray(fc1_b.reshape(KT, 128).T, dtype=np.float32)
    # ELU's "-1" folded: h = (elu(y)+1) @ w2.T + (b2 - w2.sum(axis=1))
    b2f = (fc2_b - fc2_w.sum(axis=1)).astype(np.float32)
    b2p = np.ascontiguousarray(b2f.reshape(KT, 128).T, dtype=np.float32)

    in_maps = []
    for c in range(N_CORES):
        blk1 = z1[c * NB:(c + 1) * NB].T            # [512, 1024]
        blk2 = z2[c * NB:(c + 1) * NB].T
        zt1 = _to_fp8(np.ascontiguousarray(
            blk1.reshape(KT, 128, NB).transpose(1, 0, 2)))
        zt2 = _to_fp8(np.ascontiguousarray(
            blk2.reshape(KT, 128, NB).transpose(1, 0, 2)))
        in_maps.append({"zt1": zt1, "zt2": zt2, "w1t": w1t, "w2t": w2t,
                        "b1": b1, "b2p": b2p})
    return in_maps


def kernel(z1, z2, fc1_w, fc1_b, fc2_w, fc2_b):
    global LAST_EXEC_NS
    z1 = np.asarray(z1, dtype=np.float32)
    z2 = np.asarray(z2, dtype=np.float32)
    fc1_w = np.asarray(fc1_w, dtype=np.float32)
    fc1_b = np.asarray(fc1_b, dtype=np.float32)
    fc2_w = np.asarray(fc2_w, dtype=np.float32)
    fc2_b = np.asarray(fc2_b, dtype=np.float32)

    if "nc" not in _CACHE:
        _CACHE["nc"] = _build_program()
    nc = _CACHE["nc"]

    in_maps = _prep_inputs(z1, z2, fc1_w, fc1_b, fc2_w, fc2_b)
    res = run_bass_kernel_spmd(nc, in_maps, core_ids=list(range(N_CORES)),
                               trace=TRACE)
    LAST_EXEC_NS = res.exec_time_ns
    total = math.fsum(float(r["out"][0, 0]) for r in res.results)
    return np.float32(total / N)


# revision 24
# speedup vs baseline: 1.2277x; 1.2277x over previous
"""GRACE contrastive loss kernel for Trainium2 (8 NeuronCores, SPMD).

Strategy (symmetric row-block data parallel, fp8 everywhere):
  - Shard the N=8192 nodes across 8 cores (1024 rows each).  Projection
    MLP runs in fp8 DoubleRow (weights + activations quantized, ELU's
    "-1" folded into b2 host-side), fp32 accumulation; per-node 1/norms
    via exp(-0.5*ln(sum h^2)); normalized embeddings quantized to fp8
    and AllGather'd per view (a tiny dummy AllGather issued at t=0
    pulls the collective entry barrier off the critical path).
  - S11/S22 are symmetric: each core computes only shifts d=0..4 of its
    block row (d = (col_block - core) mod 8); row sums of the computed
    exp-blocks cover d=0..4, the d=5..7 contributions arrive as column
    sums computed by neighbor cores (shift 8-d in {1,2,3}), routed via
    one fp32 ReduceScatter.  The d=4 block is computed redundantly by
    both cores of its pair (no colsum exchange) to keep the SPMD
    program uniform.  S12 is computed in full per row; its column sums
    (= S21 row sums) ride the same ReduceScatter.
  - The gathered embeddings are loaded into SBUF in *rotated* order
    (slot d holds global block (core+d) mod 8) using dynamic-offset
    DMAs driven by the partition id, so all matmul addressing is
    uniform across cores.  Shift-0 (diagonal) blocks use the local
    embeddings directly and run while the AllGathers are in flight.
  - Sim groups are fp8 DoubleRow matmuls (N=1024 moving operand) into
    [128 x 2048] (or 1024) PSUM groups with fused exp(2s) + row-sum
    (accum_out) on the scalar engine; exp tiles for colsum-contributing
    blocks accumulate into fp16 buffers (DVE), partition-reduced by
    ones-matmuls and scattered into the ReduceScatter input at
    dynamically-computed (core-relative) offsets.  All Ln's run at the
    very end so the ACT exp table is loaded only once per set.
  - Per-core scalar partial out; host sums partials / N.
"""

import math
import sys

import numpy as np

sys.path.insert(0, "/opt/trn_rl_repo")

import concourse.bass as bass  # noqa: E402
import concourse.mybir as mybir  # noqa: E402
import concourse.tile as tile  # noqa: E402
from concourse import bacc  # noqa: E402
from concourse.bass_utils import run_bass_kernel_spmd  # noqa: E402

F32 = mybir.dt.float32
F32R = mybir.dt.float32r
F16 = mybir.dt.float16
F8 = mybir.dt.float8e4
AF = mybir.ActivationFunctionType
ALU = mybir.AluOpType
DR = mybir.MatmulPerfMode.DoubleRow

N_CORES = 8
N = 8192
D = 512            # feature dim (= H = P in the reference MLP)
NB = N // N_CORES  # 1024 rows per core
KT = D // 128      # 4 k-subtiles
MT = NB // 128     # 8 row tiles per core
NCHUNK = 512       # projection matmul moving width
SIMW = 1024        # sim matmul moving width (max for fp8)
BLK = D * NB       # elements in one gathered fp8 block
TAU_INV = 2.0      # 1 / tau
E2 = float(np.exp(2.0, dtype=np.float64))

TRACE = False
LAST_EXEC_NS = None
_CACHE = {}


def _build_program(sim_mode=False):
    nc = bacc.Bacc("TRN2", target_bir_lowering=False, debug=False,
                   num_devices=N_CORES)

    # ---- I/O ----
    zt1 = nc.dram_tensor("zt1", [128, KT, NB], F8, kind="ExternalInput").ap()
    zt2 = nc.dram_tensor("zt2", [128, KT, NB], F8, kind="ExternalInput").ap()
    w1t = nc.dram_tensor("w1t", [128, KT, D], F8, kind="ExternalInput").ap()
    w2t = nc.dram_tensor("w2t", [128, KT, D], F8, kind="ExternalInput").ap()
    b1 = nc.dram_tensor("b1", [128, KT], F32, kind="ExternalInput").ap()
    b2p = nc.dram_tensor("b2p", [128, KT], F32, kind="ExternalInput").ap()
    out = nc.dram_tensor("out", [1, 1], F32, kind="ExternalOutput").ap()

    rg = [list(range(N_CORES))]

    with tile.TileContext(nc) as tc:
        with tc.tile_pool(name="persist", bufs=1) as persist, \
             tc.tile_pool(name="dram", bufs=1, space="DRAM") as dram, \
             tc.tile_pool(name="stats", bufs=1) as stats:

            ones_cs = persist.tile([128, 1], F32)
            nc.vector.memset(ones_cs[:], 1.0)
            ones_col = persist.tile([128, 1], F32R)
            nc.vector.tensor_copy(ones_col[:], ones_cs[:])
            ones_sc = persist.tile([1, 128], F32)
            nc.vector.memset(ones_sc[:], 1.0)
            ones_row = persist.tile([1, 128], F32R)
            nc.vector.tensor_copy(ones_row[:], ones_sc[:])
            ones_16 = persist.tile([128, 1], F16)
            nc.vector.memset(ones_16[:], 1.0)

            # normalized fp8 local blocks [feature, node] (sims lhsT + d0 rhs)
            n8 = [persist.tile([128, KT, NB], F8, name=f"n8_{v}")
                  for v in range(2)]
            rn_vec = [persist.tile([1, NB], F32R, name=f"rn{v}") for v in range(2)]
            # rotated gathered embeddings: slot di holds global block
            # (core + di + 1) mod 8; [128, slot, KT, NB] keeps every
            # DMA line contiguous (4KB per partition per slot)
            g1 = persist.tile([128, 4, KT, NB], F8, name="g1")
            g2 = persist.tile([128, 7, KT, NB], F8, name="g2")
            # colsum accumulators (rotated slot order)
            acc11 = persist.tile([128, 3 * NB], F16, name="acc11")
            acc22 = persist.tile([128, 3 * NB], F16, name="acc22")
            acc12 = persist.tile([128, 8 * NB], F16, name="acc12")

            # DRAM buffers
            shr = {} if sim_mode else {"addr_space": "Shared"}
            db_in = dram.tile([1, 8], F32, name="db_in")
            db_out = dram.tile([1, 64], F32, name="db_out",
                               tag="dbbuf", **shr)
            cc_in = dram.tile([2 * 128, KT * NB], F8, name="cc_in")
            ccf = dram.tile([1, N_CORES * 2 * BLK], F8, name="cc_out",
                            tag="agbuf0", **shr)
            cs_in = dram.tile([1, N_CORES * 3 * NB], F32, name="cs_in")
            cs_out = dram.tile([3 * NB], F32, name="cs_out")

            pos_part = stats.tile([1, NB], F32, name="pos_part")
            # row-sum partials: [128, MT, slots]
            parts11 = stats.tile([128, MT, 4], F32, name="parts11")
            parts12 = stats.tile([128, MT, 4], F32, name="parts12")
            parts22 = stats.tile([128, MT, 4], F32, name="parts22")
            pos_sum = stats.tile([1, 1], F32)
            lnwarm = stats.tile([1, 1], F32, name="lnwarm")

            # dummy collective: absorbs the entry barrier + first-collective
            # stream costs so AG1 runs fast (measured 24us vs 33us without)
            zcs = persist.tile([1, 3 * NB], F32, name="zcs")
            nc.vector.memset(zcs[:], 0.0)
            nc.sync.dma_start(db_in[:], zcs[:, 0:8])
            if not sim_mode:
                nc.gpsimd.collective_compute(
                    "AllGather", ALU.bypass, replica_groups=rg,
                    ins=[db_in.opt()], outs=[db_out.opt()])
            # zero-init the ReduceScatter input (S11/S22 sections of
            # slots this core does not write must contribute zero)
            for j in range(N_CORES):
                nc.sync.dma_start(cs_in[:, j * 3 * NB:(j + 1) * 3 * NB], zcs[:])

            # ---------------- projection phase ----------------
            with tc.tile_pool(name="proj", bufs=1) as proj, \
                 tc.tile_pool(name="ptmp", bufs=2) as ptmp, \
                 tc.tile_pool(name="ppsum", bufs=4, space="PSUM") as ppsum, \
                 tc.tile_pool(name="spsum", bufs=2, space="PSUM") as spsum:

                zt_sb = [proj.tile([128, KT, NB], F8, name=f"zt_sb{v}")
                         for v in range(2)]
                w1_sb = proj.tile([128, KT, D], F8)
                w2_sb = proj.tile([128, KT, D], F8)
                b1_sb = proj.tile([128, KT], F32)
                b2_sb = proj.tile([128, KT], F32)
                u_sb = proj.tile([128, KT, NB], F8)   # ELU out + 1
                h_sb = [proj.tile([128, KT, NB], F32, name=f"h{v}")
                        for v in range(2)]
                hsq = proj.tile([128, KT, NB], F32R)

                nc.sync.dma_start(w1_sb[:], w1t)
                nc.sync.dma_start(b1_sb[:], b1)
                nc.sync.dma_start(zt_sb[0][:], zt1)
                nc.sync.dma_start(w2_sb[:], w2t)
                nc.sync.dma_start(b2_sb[:], b2p)
                nc.sync.dma_start(zt_sb[1][:], zt2)

                for v in range(2):
                    # ---- layer 1 + ELU (u = elu(y) + 1 >= 0) ----
                    for pt in range(KT):
                        for ch in range(NB // NCHUNK):
                            csl = slice(ch * NCHUNK, (ch + 1) * NCHUNK)
                            ps = ppsum.tile([128, NCHUNK], F32, tag="ps_proj")
                            for k2 in range(KT // 2):
                                nc.tensor.matmul(
                                    ps[:],
                                    lhsT=w1_sb[:, 2 * k2:2 * k2 + 2,
                                               pt * 128:(pt + 1) * 128],
                                    rhs=zt_sb[v][:, 2 * k2:2 * k2 + 2, csl],
                                    start=(k2 == 0), stop=(k2 == KT // 2 - 1),
                                    perf_mode=DR)
                            texp = ptmp.tile([128, NCHUNK], F16, tag="texp")
                            nc.scalar.activation(texp[:], ps[:], AF.Exp,
                                                 bias=b1_sb[:, pt:pt + 1],
                                                 scale=1.0)
                            tmax = ptmp.tile([128, NCHUNK], F16, tag="tmax")
                            nc.scalar.activation(tmax[:], ps[:], AF.Relu,
                                                 bias=b1_sb[:, pt:pt + 1],
                                                 scale=1.0)
                            # u = min(exp(y),1) + relu(y)
                            nc.vector.scalar_tensor_tensor(
                                u_sb[:, pt, csl], texp[:], 1.0, tmax[:],
                                ALU.min, ALU.add)
                    # ---- layer 2 (+ folded b2) + squares ----
                    for jt in range(KT):
                        for ch in range(NB // NCHUNK):
                            csl = slice(ch * NCHUNK, (ch + 1) * NCHUNK)
                            ps = ppsum.tile([128, NCHUNK], F32, tag="ps_proj")
                            for k2 in range(KT // 2):
                                nc.tensor.matmul(
                                    ps[:],
                                    lhsT=w2_sb[:, 2 * k2:2 * k2 + 2,
                                               jt * 128:(jt + 1) * 128],
                                    rhs=u_sb[:, 2 * k2:2 * k2 + 2, csl],
                                    start=(k2 == 0), stop=(k2 == KT // 2 - 1),
                                    perf_mode=DR)
                            sl = (slice(None), jt, csl)
                            nc.vector.tensor_scalar(h_sb[v][sl], ps[:],
                                                    b2_sb[:, jt:jt + 1], None,
                                                    ALU.add)
                            nc.scalar.activation(hsq[sl], h_sb[v][sl], AF.Square)
                    # ---- 1/norm: rn = exp(-0.5*ln(ss)); Ln's batched ----
                    tlns = []
                    for ch in range(NB // NCHUNK):
                        csl = slice(ch * NCHUNK, (ch + 1) * NCHUNK)
                        pss = spsum.tile([1, NCHUNK], F32, tag="ps_small")
                        for jt in range(KT):
                            nc.tensor.matmul(
                                pss[:],
                                lhsT=ones_col[:],
                                rhs=hsq[:, jt, csl],
                                start=(jt == 0), stop=(jt == KT - 1))
                        tln = ptmp.tile([1, NCHUNK], F32, tag="tln")
                        nc.scalar.activation(tln[:], pss[:], AF.Ln)
                        tlns.append(tln)
                    for ch in range(NB // NCHUNK):
                        csl = slice(ch * NCHUNK, (ch + 1) * NCHUNK)
                        nc.scalar.activation(rn_vec[v][:, csl], tlns[ch][:],
                                             AF.Exp, scale=-0.5)
                    # ---- normalize + quantize to fp8 ----
                    for ch in range(NB // NCHUNK):
                        csl = slice(ch * NCHUNK, (ch + 1) * NCHUNK)
                        pbc = spsum.tile([128, NCHUNK], F32, tag="ps_bc")
                        nc.tensor.matmul(
                            pbc[:], lhsT=ones_row[:],
                            rhs=rn_vec[v][:, csl],
                            start=True, stop=True)
                        for jt in range(KT):
                            nc.vector.tensor_tensor(
                                n8[v][:, jt, csl], h_sb[v][:, jt, csl],
                                pbc[:], ALU.mult)
                    # ---- ship; one merged AllGather after both views ----
                    for hf in range(2):
                        nc.sync.dma_start(
                            cc_in[v * 128:(v + 1) * 128,
                                  hf * 2 * NB:(hf + 1) * 2 * NB],
                            n8[v][:, 2 * hf:2 * hf + 2, :])
                    if v == 1:
                        if sim_mode:
                            for r in range(N_CORES):
                                nc.sync.dma_start(
                                    ccf[:, r * 2 * BLK:(r + 1) * 2 * BLK],
                                    cc_in.rearrange("p m -> (p m)"))
                        else:
                            nc.gpsimd.collective_compute(
                                "AllGather", ALU.bypass, replica_groups=rg,
                                ins=[cc_in.opt()], outs=[ccf.opt()])

                # ---- pos diagonal: s12_ii = rn1_i*rn2_i*sum_f h1[f,i]h2[f,i]
                hh = hsq  # reuse
                for jt in range(KT):
                    nc.vector.tensor_tensor(hh[:, jt, :], h_sb[0][:, jt, :],
                                            h_sb[1][:, jt, :], ALU.mult)
                for ch in range(NB // NCHUNK):
                    csl = slice(ch * NCHUNK, (ch + 1) * NCHUNK)
                    psp = spsum.tile([1, NCHUNK], F32, tag="ps_small")
                    for jt in range(KT):
                        nc.tensor.matmul(psp[:],
                                         lhsT=ones_col[:],
                                         rhs=hh[:, jt, csl],
                                         start=(jt == 0), stop=(jt == KT - 1))
                    nc.vector.tensor_tensor(pos_part[:, csl], psp[:],
                                            rn_vec[0][:, csl], ALU.mult)
                    nc.vector.tensor_tensor(pos_part[:, csl], pos_part[:, csl],
                                            rn_vec[1][:, csl], ALU.mult)
                nc.vector.tensor_reduce(pos_sum[:], pos_part[:],
                                        mybir.AxisListType.X, ALU.add)

            # ---------------- rotated gathered loads ----------------
            pid = nc.sync.partition_id()
            for d in range(1, 5):
                off = ((pid + d) % N_CORES) * (2 * BLK)
                nc.sync.dma_start(
                    g1[:, d - 1, :, :],
                    ccf[:, bass.ds(off, BLK)]
                    .rearrange("o (p m) -> (o p) m", p=128))
            for d in range(1, 8):
                off = ((pid + d) % N_CORES) * (2 * BLK) + BLK
                nc.sync.dma_start(
                    g2[:, d - 1, :, :],
                    ccf[:, bass.ds(off, BLK)]
                    .rearrange("o (p m) -> (o p) m", p=128))

            # ---------------- similarity passes ----------------
            # chunks: list of (lhs_view, rhs3d) where rhs3d is [128, KT, NB]
            # flush: (acc, aoff, sec, d) — partition-reduce one NB-wide acc
            # column into the unused upper half of a 1024-wide group's psum
            # slot, then scatter it into cs_in at the core-relative slot.
            def sim_group(sp, scr, cs_sbp, mt, width, chunks, parts_ap,
                          acc_list, first_touch, dve_rowsum=False, flush=None):
                pss = sp.tile([128, 2048], F32, tag="ps_sim")
                for ci, (lv, r3) in enumerate(chunks):
                    for k2 in range(KT // 2):
                        for sc in range(SIMW // NCHUNK):
                            c0 = sc * NCHUNK
                            o0 = ci * SIMW + sc * NCHUNK
                            nc.tensor.matmul(
                                pss[:, o0:o0 + NCHUNK],
                                lhsT=n8[lv][:, 2 * k2:2 * k2 + 2,
                                            mt * 128:(mt + 1) * 128],
                                rhs=r3[:, 2 * k2:2 * k2 + 2, c0:c0 + NCHUNK],
                                start=(k2 == 0), stop=(k2 == KT // 2 - 1),
                                perf_mode=DR)
                if flush is not None:
                    facc, aoff, sec, dd = flush
                    for sc in range(NB // NCHUNK):
                        nc.tensor.matmul(
                            pss[0:1, 1024 + sc * NCHUNK:1024 + (sc + 1) * NCHUNK],
                            lhsT=ones_16[:],
                            rhs=facc[:, aoff + sc * NCHUNK:aoff + (sc + 1) * NCHUNK],
                            start=True, stop=True)
                    cst = cs_sbp.tile([1, NB], F32, tag="cs")
                    nc.vector.tensor_copy(cst[:], pss[0:1, 1024:2048])
                    woff = ((pid + dd) % N_CORES) * (3 * NB) + sec * NB
                    nc.sync.dma_start(cs_in[:, bass.ds(woff, NB)], cst[:])
                es = scr.tile([128, width], F16,
                              tag=("es2" if width == 2048 else "es1"))
                if dve_rowsum:
                    nc.scalar.activation(es[:], pss[:, 0:width], AF.Exp,
                                         scale=TAU_INV)
                    nc.vector.tensor_reduce(parts_ap, es[:],
                                            mybir.AxisListType.X, ALU.add)
                else:
                    nc.scalar.activation(es[:], pss[:, 0:width], AF.Exp,
                                         scale=TAU_INV, accum_out=parts_ap)
                for (acc, aoff, eoff, w2) in acc_list:
                    asl = acc[:, aoff:aoff + w2]
                    esl = es[:, eoff:eoff + w2]
                    if first_touch:
                        nc.vector.tensor_copy(asl, esl)
                    else:
                        nc.vector.tensor_tensor(asl, asl, esl, ALU.add)

            with tc.tile_pool(name="sim_psum", bufs=2, space="PSUM") as sp, \
                 tc.tile_pool(name="scr", bufs=4) as scr, \
                 tc.tile_pool(name="cs_sbp", bufs=4) as cs_sbp:

                # P1: shift-0 diagonal blocks (local, overlap AllGathers).
                # [S11d0|S12d0] share one group (both feed denom1 rows);
                # row sums on DVE (idle during the AllGather window).
                for mt in range(MT):
                    ft = (mt == 0)
                    sim_group(sp, scr, cs_sbp, mt, 2048,
                              [(0, n8[0]), (0, n8[1])],
                              parts11[:, mt, 0:1], [(acc12, 0, NB, NB)], ft,
                              dve_rowsum=True)
                    sim_group(sp, scr, cs_sbp, mt, 1024, [(1, n8[1])],
                              parts22[:, mt, 0:1], [], ft, dve_rowsum=True)

                # P2: S11 shifts 1..3 (needs g1)
                for mt in range(MT):
                    ft = (mt == 0)
                    sim_group(sp, scr, cs_sbp, mt, 2048,
                              [(0, g1[:, 0]), (0, g1[:, 1])],
                              parts11[:, mt, 1:2], [(acc11, 0, 0, 2 * NB)], ft)
                    sim_group(sp, scr, cs_sbp, mt, 1024, [(0, g1[:, 2])],
                              parts11[:, mt, 2:3], [(acc11, 2 * NB, 0, NB)], ft)

                # P3: S12 shifts 1..7 (needs g2); acc11 colsums flush inside
                # the d7 groups' psum upper halves
                for mt in range(MT):
                    ft = (mt == 0)
                    sim_group(sp, scr, cs_sbp, mt, 2048,
                              [(0, g2[:, 0]), (0, g2[:, 1])],
                              parts12[:, mt, 0:1], [(acc12, NB, 0, 2 * NB)], ft)
                    sim_group(sp, scr, cs_sbp, mt, 2048,
                              [(0, g2[:, 2]), (0, g2[:, 3])],
                              parts12[:, mt, 1:2],
                              [(acc12, 3 * NB, 0, 2 * NB)], ft)
                    sim_group(sp, scr, cs_sbp, mt, 2048,
                              [(0, g2[:, 4]), (0, g2[:, 5])],
                              parts12[:, mt, 2:3],
                              [(acc12, 5 * NB, 0, 2 * NB)], ft)
                    fl = ((acc11, (mt - 1) * NB, 1, mt) if 1 <= mt <= 3
                          else None)
                    sim_group(sp, scr, cs_sbp, mt, 1024, [(0, g2[:, 6])],
                              parts12[:, mt, 3:4], [(acc12, 7 * NB, 0, NB)],
                              ft, flush=fl)

                # P4: S22 shifts 1..3; acc12 colsums flush inside d3 groups
                for mt in range(MT):
                    ft = (mt == 0)
                    sim_group(sp, scr, cs_sbp, mt, 2048,
                              [(1, g2[:, 0]), (1, g2[:, 1])],
                              parts22[:, mt, 1:2], [(acc22, 0, 0, 2 * NB)], ft)
                    sim_group(sp, scr, cs_sbp, mt, 1024, [(1, g2[:, 2])],
                              parts22[:, mt, 2:3], [(acc22, 2 * NB, 0, NB)],
                              ft, flush=(acc12, mt * NB, 0, mt))

                # P5: redundant shift-4 blocks (no colsums); acc22 flushes in
                # the first three groups, then the ReduceScatter fires and
                # overlaps the rest of the pass
                for mt in range(MT):
                    fl = (acc22, (mt) * NB, 2, mt + 1) if mt <= 2 else None
                    sim_group(sp, scr, cs_sbp, mt, 1024, [(0, g1[:, 3])],
                              parts11[:, mt, 3:4], [], False,
                              dve_rowsum=True, flush=fl)
                    sim_group(sp, scr, cs_sbp, mt, 1024, [(1, g2[:, 3])],
                              parts22[:, mt, 3:4], [], False, dve_rowsum=True)
                    if mt == 3:
                        if sim_mode:
                            nc.sync.dma_start(cs_out[:],
                                              cs_in[:, 0:3 * NB]
                                              .rearrange("o m -> (o m)"))
                        else:
                            nc.gpsimd.collective_compute(
                                "ReduceScatter", ALU.add, replica_groups=rg,
                                ins=[cs_in.opt()], outs=[cs_out.opt()])
                # warm the Ln table set while the ReduceScatter drains, so
                # the final assembly pays no ACT_TABLE_LOAD.  The warm input
                # is parts22*0+1 so it depends on the last sim exp (not
                # hoisted) and ln(1)=0 folds harmlessly into pos_sum.
                nc.vector.tensor_scalar(lnwarm[:], parts22[0:1, 7, 3:4],
                                        0.0, 1.0, ALU.mult, ALU.add)
                nc.scalar.activation(lnwarm[:], lnwarm[:], AF.Ln)
                nc.vector.tensor_tensor(pos_sum[:], pos_sum[:], lnwarm[:],
                                        ALU.add)

            # ---------------- assemble the loss ----------------
            with tc.tile_pool(name="fin", bufs=1) as fsb, \
                 tc.tile_pool(name="fin_psum", bufs=1, space="PSUM") as fp:
                rs11 = fsb.tile([128, MT], F32)
                rs12 = fsb.tile([128, MT], F32)
                rs22 = fsb.tile([128, MT], F32)
                nc.vector.tensor_reduce(rs11[:], parts11[:],
                                        mybir.AxisListType.X, ALU.add)
                nc.vector.tensor_reduce(rs12[:], parts12[:],
                                        mybir.AxisListType.X, ALU.add)
                nc.vector.tensor_reduce(rs22[:], parts22[:],
                                        mybir.AxisListType.X, ALU.add)
                r21 = fsb.tile([128, MT], F32)
                r11 = fsb.tile([128, MT], F32)
                r22 = fsb.tile([128, MT], F32)
                nc.sync.dma_start(
                    r21[:], cs_out[0:NB].rearrange("(mt p) -> p mt", p=128))
                nc.sync.dma_start(
                    r11[:], cs_out[NB:2 * NB].rearrange("(mt p) -> p mt", p=128))
                nc.sync.dma_start(
                    r22[:], cs_out[2 * NB:3 * NB]
                    .rearrange("(mt p) -> p mt", p=128))

                d1 = fsb.tile([128, MT], F32)
                nc.vector.tensor_tensor(d1[:], rs11[:], r11[:], ALU.add)
                nc.vector.tensor_tensor(d1[:], d1[:], rs12[:], ALU.add)
                nc.vector.tensor_scalar_add(d1[:], d1[:], -E2)
                d2 = fsb.tile([128, MT], F32)
                nc.vector.tensor_tensor(d2[:], rs22[:], r22[:], ALU.add)
                nc.vector.tensor_tensor(d2[:], d2[:], r21[:], ALU.add)
                nc.vector.tensor_scalar_add(d2[:], d2[:], -E2)
                nc.scalar.activation(d1[:], d1[:], AF.Ln)
                nc.scalar.activation(d2[:], d2[:], AF.Ln)
                lsum = fsb.tile([128, MT], F32)
                nc.vector.tensor_tensor(lsum[:], d1[:], d2[:], ALU.add)
                lrow = fsb.tile([128, 1], F32)
                nc.vector.tensor_reduce(lrow[:], lsum[:],
                                        mybir.AxisListType.X, ALU.add)
                pfin = fp.tile([1, 1], F32)
                nc.tensor.matmul(pfin[:], lhsT=ones_cs[:], rhs=lrow[:],
                                 start=True, stop=True)
                fin = fsb.tile([1, 1], F32)
                nc.vector.tensor_scalar_mul(fin[:], pfin[:], 0.5)
                p2 = fsb.tile([1, 1], F32)
                nc.vector.tensor_scalar_mul(p2[:], pos_sum[:], 2.0)
                nc.vector.tensor_tensor(fin[:], fin[:], p2[:], ALU.subtract)
                nc.sync.dma_start(out, fin[:])

    nc.compile()
    return nc


def _to_fp8(x):
    import ml_dtypes
    return np.asarray(x, dtype=ml_dtypes.float8_e4m3fn)


def _prep_inputs(z1, z2, fc1_w, fc1_b, fc2_w, fc2_b):
    """Host-side shard + layout prep. Returns in_maps for the 8 cores."""
    w1t = np.ascontiguousarray(fc1_w.T).reshape(KT, 128, D).transpose(1, 0, 2)
    w1t = _to_fp8(np.ascontiguousarray(w1t))
    w2t = np.ascontiguousarray(fc2_w.T).reshape(KT, 128, D).transpose(1, 0, 2)
    w2t = _to_fp8(np.ascontiguousarray(w2t))
    b1 = np.ascontiguousarray(fc1_b.reshape(KT, 128).T, dtype=np.float32)
    # ELU's "-1" folded: h = (elu(y)+1) @ w2.T + (b2 - w2.sum(axis=1))
    b2f = (fc2_b - fc2_w.sum(axis=1)).astype(np.float32)
    b2p = np.ascontiguousarray(b2f.reshape(KT, 128).T, dtype=np.float32)

    in_maps = []
    for c in range(N_CORES):
        blk1 = z1[c * NB:(c + 1) * NB].T            # [512, 1024]
        blk2 = z2[c * NB:(c + 1) * NB].T
        zt1 = _to_fp8(np.ascontiguousarray(
            blk1.reshape(KT, 128, NB).transpose(1, 0, 2)))
        zt2 = _to_fp8(np.ascontiguousarray(
            blk2.reshape(KT, 128, NB).transpose(1, 0, 2)))
        in_maps.append({"zt1": zt1, "zt2": zt2, "w1t": w1t, "w2t": w2t,
                        "b1": b1, "b2p": b2p})
    return in_maps


def kernel(z1, z2, fc1_w, fc1_b, fc2_w, fc2_b):
    global LAST_EXEC_NS
    z1 = np.asarray(z1, dtype=np.float32)
    z2 = np.asarray(z2, dtype=np.float32)
    fc1_w = np.asarray(fc1_w, dtype=np.float32)
    fc1_b = np.asarray(fc1_b, dtype=np.float32)
    fc2_w = np.asarray(fc2_w, dtype=np.float32)
    fc2_b = np.asarray(fc2_b, dtype=np.float32)

    if "nc" not in _CACHE:
        _CACHE["nc"] = _build_program()
    nc = _CACHE["nc"]

    in_maps = _prep_inputs(z1, z2, fc1_w, fc1_b, fc2_w, fc2_b)
    res = run_bass_kernel_spmd(nc, in_maps, core_ids=list(range(N_CORES)),
                               trace=TRACE)
    LAST_EXEC_NS = res.exec_time_ns
    total = math.fsum(float(r["out"][0, 0]) for r in res.results)
    return np.float32(total / N)
